# revision 5
# baseline (speedup 1.0000x reference)
"""Dense multi-head attention (S=4096, H=16, D=64) on 8 Trainium2 NeuronCores.

Sharding: heads split across cores (2 heads per core), no cross-core comms.

Per-core kernel (per head):
  - Load Q, K natural-layout, PE-transpose to QT/KT [64, 4096] (d on partitions).
  - Load V with an appended ones-column -> V' [128, 65] per k-tile.
  - For each 512-wide q chunk: S^T tiles [128 k, 512 q] = KT_tile.T @ QT_chunk
    (f32r matmuls, 1 cycle/row), exp via ScalarE with scale=1/8 fused
    (softmax without max-subtract: logits ~ N(0,1), no overflow possible),
    then O'^T [65, 512] += V'_tile.T @ E accumulated over all 32 k-tiles.
    Row 64 of O'^T is the softmax denominator (ones-column trick).
  - Epilogue: PE-transpose O'^T back to [128 q, 65], reciprocal of col 64,
    per-row scale, DMA out.
"""

import numpy as np

import concourse.bass as bass
import concourse.mybir as mybir
import concourse.tile as tile
from concourse import bacc
from concourse.bass_utils import run_bass_kernel_spmd
from concourse.masks import make_identity

S = 4096
H = 16
D = 64
NCORES = 8
HPC = H // NCORES  # heads per core
NKT = S // 128  # 32 k-tiles per head
NQC = S // 512  # 8 q chunks per head
SCALE = 1.0 / np.sqrt(D)

F32 = mybir.dt.float32
F32R = mybir.dt.float32r


def r(ap):
    """Bitcast an fp32 AP to float32r so the PE streams it at 1 cycle/row."""
    return ap.bitcast(F32R)


def _build_head(nc, tc, pools, idn, q, k, v, o, h):
    sb, cpool, epool, spsum, opsum, tpsum = pools

    # ---- Phase A: load + transpose Q,K; load V with ones column ----
    qstage = sb.tile([128, NKT, D], F32, tag="qstage")
    kstage = sb.tile([128, NKT, D], F32, tag="kstage")
    nc.sync.dma_start(qstage[:], q.ap()[h].rearrange("(n p) d -> p n d", p=128))
    nc.sync.dma_start(kstage[:], k.ap()[h].rearrange("(n p) d -> p n d", p=128))

    vstage = sb.tile([128, NKT, D + 1], F32R, tag="vstage")
    nc.sync.dma_start(
        vstage[:, :, 0:D], v.ap()[h].rearrange("(n p) d -> p n d", p=128).bitcast(F32R)
    )
    ones = sb.tile([128, NKT], F32, tag="ones")
    nc.gpsimd.memset(ones[:], 1.0)
    nc.vector.tensor_copy(vstage[:, :, D], ones[:])

    qt = sb.tile([D, S], F32R, tag="qt")
    kt = sb.tile([D, S], F32R, tag="kt")
    for dst, stage, tg in ((qt, qstage, "qtp"), (kt, kstage, "ktp")):
        for g in range(NKT // 4):  # groups of 4 tiles -> one psum bank
            tp = tpsum.tile([D, 512], F32, tag="tp")
            for j in range(4):
                t = g * 4 + j
                nc.tensor.matmul(
                    tp[:, j * 128 : (j + 1) * 128],
                    stage[:, t, :],
                    idn[:, 0:128],
                    is_transpose=True,
                )
            nc.vector.tensor_copy(dst[:, g * 512 : (g + 1) * 512], tp[:])

    # ---- Phase B: attention ----
    for qc in range(NQC):
        qs = qc * 512
        acc = opsum.tile([D + 1, 512], F32, tag="acc")
        for tt in range(NKT // 2):  # pairs of k-tiles
            sp = spsum.tile([128, 1024], F32, tag="sp")
            for j in range(2):
                t = 2 * tt + j
                nc.tensor.matmul(
                    sp[:, j * 512 : (j + 1) * 512],
                    kt[:, t * 128 : (t + 1) * 128],
                    qt[:, qs : qs + 512],
                )
            et = epool.tile([128, 1024], F32R, tag="et")
            nc.scalar.activation(
                et[:], sp[:], mybir.ActivationFunctionType.Exp, scale=SCALE
            )
            for j in range(2):
                t = 2 * tt + j
                nc.tensor.matmul(
                    acc[:],
                    vstage[:, t, :],
                    et[:, j * 512 : (j + 1) * 512],
                    start=(t == 0),
                    stop=(t == NKT - 1),
                )

        # ---- epilogue for this q chunk ----
        ot = sb.tile([D + 1, 512], F32, tag="ot")
        nc.vector.tensor_copy(ot[:], acc[:])
        fin = sb.tile([128, 4, D], F32, tag="fin")
        for j in range(4):
            tp2 = tpsum.tile([128, D + 1], F32, tag="tp")
            nc.tensor.matmul(
                tp2[:],
                ot[:, j * 128 : (j + 1) * 128],
                idn[0 : D + 1, 0 : D + 1],
                is_transpose=True,
            )
            rcp = sb.tile([128, 1], F32, tag="rcp")
            nc.vector.reciprocal(rcp[:], tp2[:, D : D + 1])
            nc.vector.tensor_scalar_mul(fin[:, j, :], tp2[:, 0:D], rcp[:])
        nc.sync.dma_start(
            o.ap()[h, qs : qs + 512, :].rearrange("(n p) d -> p n d", p=128),
            fin[:],
        )


def _build():
    nc = bacc.Bacc(trn_type="TRN2", debug=False, num_devices=NCORES)
    q = nc.dram_tensor("q", [HPC, S, D], F32, kind="ExternalInput")
    k = nc.dram_tensor("k", [HPC, S, D], F32, kind="ExternalInput")
    v = nc.dram_tensor("v", [HPC, S, D], F32, kind="ExternalInput")
    o = nc.dram_tensor("o", [HPC, S, D], F32, kind="ExternalOutput")

    with tile.TileContext(nc) as tc:
        with (
            tc.tile_pool(name="const", bufs=1) as cpool,
            tc.tile_pool(name="sb", bufs=2) as sb,
            tc.tile_pool(name="epool", bufs=3) as epool,
            tc.tile_pool(name="spsum", bufs=2, space="PSUM") as spsum,
            tc.tile_pool(name="opsum", bufs=2, space="PSUM") as opsum,
            tc.tile_pool(name="tpsum", bufs=2, space="PSUM") as tpsum,
        ):
            idn = cpool.tile([128, 128], F32, tag="idn")
            make_identity(nc, idn[:])
            pools = (sb, cpool, epool, spsum, opsum, tpsum)
            for h in range(HPC):
                _build_head(nc, tc, pools, idn, q, k, v, o, h)

    nc.compile()
    return nc


_NC_CACHE = None


def kernel(query, key, value):
    global _NC_CACHE
    if _NC_CACHE is None:
        _NC_CACHE = _build()
    nc = _NC_CACHE

    in_maps = []
    for c in range(NCORES):
        sl = slice(c * HPC, (c + 1) * HPC)
        in_maps.append(
            {
                "q": np.ascontiguousarray(np.asarray(query)[:, sl, :].transpose(1, 0, 2)),
                "k": np.ascontiguousarray(np.asarray(key)[:, sl, :].transpose(1, 0, 2)),
                "v": np.ascontiguousarray(np.asarray(value)[:, sl, :].transpose(1, 0, 2)),
            }
        )

    res = run_bass_kernel_spmd(nc, in_maps, core_ids=list(range(NCORES)))
    out = np.concatenate(
        [res.results[c]["o"].transpose(1, 0, 2) for c in range(NCORES)], axis=1
    )
    return out
